# revision 2
# baseline (speedup 1.0000x reference)
"""GATv2 (2-layer, heads=1) on 8 trn2 NeuronCores — v2 kernel.

Layout/algorithm (v2):
- Nodes dealt to 8 cores by in-degree rank; SHARD=6272 slots/core; 49
  chunks of 128 lanes, chunk slot profile (K0,K1,K2) over 3 int16 gather
  windows.
- Node phase: table_e = x@(w*perm(Wl)) [NPOS,128] bf16 and
  table_r = x@(w*perm(Wr)) [NPOS,128] written per 4-tile group from one
  256-wide matmul.
- Pad rows are POISONED: pad node input columns hold q = solve(Wc_l^T, pat)
  with pat = -C on att-positive dims / +C on att-negative dims, so any
  pad-slot logit is <= -0.2*C*128 and exp underflows to 0 (replaces the mask).
- Edge phase per chunk: ut = gather(zl) + xrb (broadcast adds per window
  run), u = prelu(ut) (one ScalarE op), fold1 u[d]+u[d+64] (sign-
  homogeneous pairs by construction of perm), +- range reduces, logit,
  exp, s, r=1/s; per-slot avs = zl*av[s] (round-robin DVE/Act/Pool) and
  identity-lhsT matmul accumulation; out = Relu(psum*r) in one ScalarE
  activation (biases are all zero in this problem; asserted on host).
- Layer outputs stay in w*perm space; host folds inverses into layer-2
  weights / final output. Layer 2 exchanges activations with one
  AllGather of transposed shards.
"""

import numpy as np

SLOPE = 0.2
CORES = 8
GRP = 6   # chunks per gather-call group
CAP = 96  # slot-blocks per group (SBUF)
POISON = 512.0


class Cfg:
    def __init__(self, N, E, SHARD, W0E, W1B, W1E, W2B):
        self.N, self.E, self.SHARD = N, E, SHARD
        self.NCHUNK = SHARD // 128
        self.NPOS = CORES * SHARD
        self.W0E, self.W1B, self.W1E, self.W2B = W0E, W1B, W1E, W2B
        self.WB = [0, W1B, W2B]
        self.WE = [W0E, W1E, self.NPOS]
        npercore = [N // CORES + (1 if c < N % CORES else 0) for c in range(CORES)]
        assert all(p == npercore[0] for p in npercore), "uniform shards required"
        self.NPC = npercore[0]
        self.ZROW = []
        for w in range(3):
            z = None
            for c in range(CORES):
                r = c * SHARD + self.NPC
                if self.WB[w] <= r < self.WE[w] and self.NPC < SHARD:
                    z = r
                    break
            assert z is not None, f"no pad row inside window {w}"
            self.ZROW.append(z)


FULL = Cfg(N=50000, E=800000, SHARD=6272, W0E=32768, W1B=8704, W1E=41472, W2B=17408)
IN = 128
D = 128


# ----------------------------------------------------------------------------
# host-side graph preprocessing (same partitioning as v1)
# ----------------------------------------------------------------------------


def preprocess(edge_index, cfg=FULL):
    src = np.asarray(edge_index[0], dtype=np.int64)
    dst = np.asarray(edge_index[1], dtype=np.int64)
    N, SHARD, NCHUNK, NPOS = cfg.N, cfg.SHARD, cfg.NCHUNK, cfg.NPOS
    deg = np.bincount(dst, minlength=N)

    order = np.argsort(-deg, kind="stable")
    core_of = np.empty(N, dtype=np.int64)
    core_of[order] = np.arange(N) % CORES

    def assign_rows(key1, key2):
        row_of = np.empty(N, dtype=np.int64)
        nodemap = np.full((CORES, SHARD), -1, dtype=np.int64)
        for c in range(CORES):
            nodes = np.where(core_of == c)[0]
            k = np.lexsort((-key2[nodes], -key1[nodes]))
            nodes = nodes[k]
            row_of[nodes] = c * SHARD + np.arange(len(nodes))
            nodemap[c, : len(nodes)] = nodes
        return row_of, nodemap

    row_of, nodemap = assign_rows(deg, np.zeros(N, dtype=np.int64))
    for _ in range(2):
        src_rows = row_of[src]
        m0 = np.bincount(dst[src_rows < cfg.W1B], minlength=N)
        m2 = np.bincount(dst[src_rows >= cfg.W1E], minlength=N)
        row_of, nodemap = assign_rows(deg, m0 - m2)
    src_rows = row_of[src]
    dst_rows = row_of[dst]

    eorder = np.lexsort((src_rows, dst_rows))
    s_sorted = src_rows[eorder]
    d_sorted = dst_rows[eorder]
    starts = np.searchsorted(d_sorted, np.arange(NPOS))
    ends = np.searchsorted(d_sorted, np.arange(NPOS) + 1)

    lanes_chunk = (np.arange(NPOS) % SHARD) // 128

    K0 = np.zeros(NCHUNK, dtype=np.int64)
    K1 = np.zeros(NCHUNK, dtype=np.int64)
    K2 = np.zeros(NCHUNK, dtype=np.int64)
    for c in range(NCHUNK):
        lanes = np.where(lanes_chunk == c)[0]
        lists = [s_sorted[starts[p] : ends[p]] for p in lanes]
        degl = np.array([len(sl) for sl in lists])
        n_lt_w0e = np.array([np.searchsorted(sl, cfg.W0E) for sl in lists])
        n_lt_w1e = np.array([np.searchsorted(sl, cfg.W1E) for sl in lists])
        n_lt_w1b = np.array([np.searchsorted(sl, cfg.W1B) for sl in lists])
        n_lt_w2b = np.array([np.searchsorted(sl, cfg.W2B) for sl in lists])
        best = None
        for k0 in range(int(n_lt_w1b.max()), int(n_lt_w0e.max()) + 2):
            t0 = np.minimum(n_lt_w0e, k0)
            m1 = np.maximum(n_lt_w2b - t0, 0)
            for k1 in range(int(m1.max()), int((n_lt_w1e - t0).max()) + 2):
                t1 = np.minimum(n_lt_w1e - t0, k1)
                k2 = max(0, int((degl - t0 - t1).max()))
                if best is None or k0 + k1 + k2 < best[0]:
                    best = (k0 + k1 + k2, k0, k1, k2)
        K0[c], K1[c], K2[c] = best[1], best[2], best[3]

    K = K0 + K1 + K2
    koff = np.concatenate([[0], np.cumsum(K)])
    TK = int(koff[-1])
    idx_rows = np.zeros((CORES, TK, 128), dtype=np.int64)
    for c in range(NCHUNK):
        b = koff[c]
        idx_rows[:, b : b + K0[c], :] = cfg.ZROW[0]
        idx_rows[:, b + K0[c] : b + K0[c] + K1[c], :] = cfg.ZROW[1]
        idx_rows[:, b + K0[c] + K1[c] : koff[c + 1], :] = cfg.ZROW[2]
    for p in range(NPOS):
        e0, e1 = starts[p], ends[p]
        if e1 == e0:
            continue
        core, pos = p // SHARD, p % SHARD
        c, lane = pos // 128, pos % 128
        sl = s_sorted[e0:e1]
        t0 = min(int(np.searchsorted(sl, cfg.W0E)), int(K0[c]))
        t1 = min(int(np.searchsorted(sl, cfg.W1E)) - t0, int(K1[c]))
        t2 = len(sl) - t0 - t1
        assert t2 <= K2[c]
        b = koff[c]
        idx_rows[core, b : b + t0, lane] = sl[:t0]
        b1 = koff[c] + K0[c]
        idx_rows[core, b1 : b1 + t1, lane] = sl[t0 : t0 + t1]
        b2 = koff[c] + K0[c] + K1[c]
        idx_rows[core, b2 : b2 + t2, lane] = sl[t0 + t1 :]
    for c in range(NCHUNK):
        b = koff[c]
        assert (idx_rows[:, b : b + K0[c], :] < cfg.W0E).all()
        h1 = idx_rows[:, b + K0[c] : b + K0[c] + K1[c], :]
        assert (h1 >= cfg.W1B).all() and (h1 < cfg.W1E).all()
        h2 = idx_rows[:, b + K0[c] + K1[c] : koff[c + 1], :]
        assert (h2 >= cfg.W2B).all()

    return dict(
        row_of=row_of, nodemap=nodemap, K0=K0, K1=K1, K2=K2, K=K, koff=koff,
        idx_rows=idx_rows, cfg=cfg,
    )


# ----------------------------------------------------------------------------
# weight transforms (fold-pair permutation + poison vectors)
# ----------------------------------------------------------------------------


def transform_weights(Wl, bl, Wr, br, att, bias, in_perm=None, in_w=None):
    Wl = np.asarray(Wl, np.float64)
    Wr = np.asarray(Wr, np.float64)
    bl = np.asarray(bl, np.float64)
    br = np.asarray(br, np.float64)
    att = np.asarray(att, np.float64)
    bias = np.asarray(bias, np.float64)
    if in_perm is not None:
        scale = 1.0 / in_w[in_perm]
        Wl = Wl[in_perm, :] * scale[:, None]
        Wr = Wr[in_perm, :] * scale[:, None]
    w = np.abs(att)
    pos_dims = np.where(att >= 0)[0]
    neg_dims = np.where(att < 0)[0]
    Pp = len(pos_dims)
    a, bodd = Pp // 2, Pp % 2
    # fold-pair arrangement: position q and q+64 share sign for q != a (when
    # Pp odd, the (a, a+64) pair is mixed: pos at a, neg at 64+a).
    pos_positions = list(range(a)) + list(range(64, 64 + a)) + ([a] if bodd else [])
    allpos = set(pos_positions)
    neg_positions = [q for q in range(128) if q not in allpos]
    perm = np.empty(128, dtype=np.int64)
    perm[np.array(pos_positions, dtype=np.int64)] = pos_dims
    perm[np.array(neg_positions, dtype=np.int64)] = neg_dims
    sgn_pos = np.zeros(128, dtype=np.float64)
    sgn_pos[np.array(pos_positions, dtype=np.int64)] = 1.0
    sgn_pos[np.array(neg_positions, dtype=np.int64)] = -1.0

    def colT(W):
        return W[:, perm] * w[perm][None, :]

    def vecT(v):
        return (w * v)[perm]

    Ml = colT(Wl)
    pat = np.where(sgn_pos > 0, -POISON, POISON)

    return dict(
        Wc=np.concatenate([Ml, colT(Wr)], axis=1),
        Bp=vecT(bl + br), blp=vecT(bl), biasp=vecT(bias),
        perm=perm, w=w, Pp=Pp, a=a, bodd=bodd, sgn=sgn_pos, pat=pat,
    )


def host_transforms(params):
    t1 = transform_weights(
        params["Wl1"], params["bl1"], params["Wr1"], params["br1"],
        params["att1"], params["bias1"],
    )
    t2 = transform_weights(
        params["Wl2"], params["bl2"], params["Wr2"], params["br2"],
        params["att2"], params["bias2"], in_perm=t1["perm"], in_w=t1["w"],
    )
    for t in (t1, t2):
        assert np.abs(t["Bp"]).max() == 0 and np.abs(t["blp"]).max() == 0
        assert np.abs(t["biasp"]).max() == 0, "v2 kernel assumes zero biases"
    return t1, t2


# ----------------------------------------------------------------------------
# numpy emulator (validation; math-equivalent fp64, no folds)
# ----------------------------------------------------------------------------


def emulate(node_fts, params, pp):
    cfg = pp["cfg"]
    t1, t2 = host_transforms(params)
    nodemap = pp["nodemap"]
    nm = nodemap.reshape(-1)
    valid = nm >= 0
    x_rows = np.zeros((cfg.NPOS, IN), dtype=np.float64)
    x_rows[valid] = np.asarray(node_fts, np.float64)[nm[valid]]

    def layer(x_rows, t):
        te = x_rows @ t["Wc"][:, :128]
        tr = x_rows @ t["Wc"][:, 128:]
        te[np.array(cfg.ZROW)] = t["pat"][None, :]
        koff = pp["koff"]
        idx_rows = pp["idx_rows"]
        sgn = t["sgn"]
        out = np.zeros((cfg.NPOS, 128), dtype=np.float64)
        for core in range(CORES):
            own = core * cfg.SHARD
            for c in range(cfg.NCHUNK):
                b, e = koff[c], koff[c + 1]
                rows = own + c * 128 + np.arange(128)
                idx = idx_rows[core, b:e, :]
                xrb = tr[rows]
                v = te[idx] + xrb[None, :, :]
                u = np.where(v > 0, v, SLOPE * v)
                logit = (u * sgn[None, None, :]).sum(-1)
                av = np.exp(np.minimum(logit, 60.0))
                s = av.sum(0)
                r = 1.0 / np.maximum(s, 1e-16)
                psum = np.einsum("kl,kld->ld", av, te[idx])
                out[rows] = np.maximum(psum * r[:, None], 0.0)
        return out

    x2 = layer(x_rows, t1)
    out2 = layer(x2, t2)
    un = out2 / t2["w"][t2["perm"]][None, :]
    full = np.zeros((cfg.N, D), dtype=np.float64)
    full[nm[valid]] = un[valid][:, np.argsort(t2["perm"])]
    return full


# ----------------------------------------------------------------------------
# device program
# ----------------------------------------------------------------------------


def wrap_idx(flat):
    n = flat.shape[0]
    w = flat.reshape(n // 16, 16).T.astype(np.int16)
    return np.tile(w, (8, 1))


def make_groups(K):
    groups = []
    cur = []
    tot = 0
    for c in range(len(K)):
        k = int(K[c])
        if cur and (tot + k > CAP or len(cur) >= GRP):
            groups.append(cur)
            cur, tot = [], 0
        cur.append(c)
        tot += k
    if cur:
        groups.append(cur)
    return groups


def build_program(
    pp, t1m, t2m, rep=1, only_layer1=False, debug_out=False, sim_relu=False,
    small_gather=False, no_edge=False, no_collective=False, no_chunk=False,
):
    import concourse.bass as bass
    import concourse.mybir as mybir
    import concourse.tile as tile
    from concourse import bacc

    cfg = pp["cfg"]
    fp32, bf16, i16 = mybir.dt.float32, mybir.dt.bfloat16, mybir.dt.int16
    K0, K1, K2, K, koff = pp["K0"], pp["K1"], pp["K2"], pp["K"], pp["koff"]
    NCHUNK, SHARD, NPOS, NPC = cfg.NCHUNK, cfg.SHARD, cfg.NPOS, cfg.NPC
    TK = int(koff[-1])
    NT = NPOS // 128
    XRC = SHARD // 16

    groups = make_groups(K)
    gsz = []
    for chs in groups:
        gsz.append(
            (
                int(sum(K0[c] for c in chs)) * 128,
                int(sum(K1[c] for c in chs)) * 128,
                int(sum(K2[c] for c in chs)) * 128,
            )
        )
    i0cols = sum(n0 // 16 for n0, _, _ in gsz)
    i1cols = sum(n1 // 16 for _, n1, _ in gsz)
    i2cols = sum(n2 // 16 for _, _, n2 in gsz)

    nc = bacc.Bacc(
        "TRN2", target_bir_lowering=False, debug=False, num_devices=CORES,
        num_swdge_queues=4,
    )
    xT = nc.dram_tensor("xT", [128, NPOS], bf16, kind="ExternalInput").ap()
    W1c = nc.dram_tensor("W1c", [128, 256], bf16, kind="ExternalInput").ap()
    W2c = nc.dram_tensor("W2c", [128, 256], bf16, kind="ExternalInput").ap()
    # cons: [identity | q2-rows]
    cons = nc.dram_tensor("cons", [128, 3 * 128], bf16, kind="ExternalInput").ap()
    idx0 = nc.dram_tensor("idx0", [128, max(i0cols, 8)], i16, kind="ExternalInput").ap()
    idx1 = nc.dram_tensor("idx1", [128, max(i1cols, 8)], i16, kind="ExternalInput").ap()
    idx2 = nc.dram_tensor("idx2", [128, max(i2cols, 8)], i16, kind="ExternalInput").ap()
    xidx = nc.dram_tensor("xidx", [128, 3 * XRC], i16, kind="ExternalInput").ap()
    table_e = nc.dram_tensor("table_e", [NPOS, 128], bf16)
    table_r = nc.dram_tensor("table_r", [NPOS, 128], bf16)
    x2s = nc.dram_tensor("x2s", [SHARD, 128], bf16)
    x2t_d = nc.dram_tensor("x2t_d", [128, SHARD], bf16)
    ag = nc.dram_tensor("ag", [CORES * 128, SHARD], bf16, addr_space="Shared")
    out_ext = nc.dram_tensor("out", [SHARD, 128], fp32, kind="ExternalOutput").ap()
    if debug_out:
        dbg_x2 = nc.dram_tensor(
            "dbg_x2", [SHARD, 128], bf16, kind="ExternalOutput"
        ).ap()
        dbg_te = nc.dram_tensor(
            "dbg_te", [NPOS, 128], bf16, kind="ExternalOutput"
        ).ap()

    with tile.TileContext(nc) as tc:
        with (
            tc.tile_pool(name="res", bufs=1) as res,
            tc.tile_pool(name="xr3", bufs=1) as xr3,
            tc.tile_pool(name="gsb", bufs=2) as gsb,
            tc.tile_pool(name="csb", bufs=4) as csb,
            tc.tile_pool(name="nsb", bufs=3) as nsb,
            tc.tile_pool(name="ps", bufs=2, space="PSUM") as ps,
            tc.tile_pool(name="ps2", bufs=2, space="PSUM") as ps2,
        ):
            i0_sb = res.tile([128, max(i0cols, 8)], i16, tag="i0")
            i1_sb = res.tile([128, max(i1cols, 8)], i16, tag="i1")
            i2_sb = res.tile([128, max(i2cols, 8)], i16, tag="i2")
            xi_sb = res.tile([128, 3 * XRC], i16, tag="xi")
            co_sb = res.tile([128, 3 * 128], bf16, tag="co")
            w1_sb = res.tile([128, 256], bf16, tag="w1")
            w2_sb = res.tile([128, 256], bf16, tag="w2")
            nc.sync.dma_start(out=i0_sb[:], in_=idx0[:])
            nc.sync.dma_start(out=i1_sb[:], in_=idx1[:])
            nc.sync.dma_start(out=i2_sb[:], in_=idx2[:])
            nc.sync.dma_start(out=xi_sb[:], in_=xidx[:])
            nc.sync.dma_start(out=co_sb[:], in_=cons[:])
            nc.sync.dma_start(out=w1_sb[:], in_=W1c[:])
            nc.sync.dma_start(out=w2_sb[:], in_=W2c[:])
            ident = co_sb[:, 0:128]
            pat_rows = [co_sb[:, 128:256], co_sb[:, 256:384]]

            copy_rr = [nc.vector, nc.scalar]
            # SWDGE queue must match Tile's round-robin DMASW lane (%4)
            qctr = [0]

            def next_q():
                if sim_relu:
                    return 0
                q = qctr[0] % 4
                qctr[0] += 1
                return q

            def node_phase(layer):
                w_sb = w1_sb if layer == 1 else w2_sb
                for q in range((NT + 3) // 4):
                    tiles = [t for t in range(q * 4, min(q * 4 + 4, NT))]
                    nq = len(tiles)
                    lhs = nsb.tile([128, nq * 128], bf16, tag="lhs")
                    if layer == 1:
                        nc.sync.dma_start(
                            out=lhs[:],
                            in_=xT[:, tiles[0] * 128 : tiles[0] * 128 + nq * 128],
                        )
                    else:
                        for j, t in enumerate(tiles):
                            o = (t * 128) // SHARD
                            p0 = t * 128 - o * SHARD
                            nc.sync.dma_start(
                                out=lhs[:, j * 128 : (j + 1) * 128],
                                in_=ag[o * 128 : (o + 1) * 128, p0 : p0 + 128],
                            )
                    pt = ps.tile([128, nq * 256], fp32, tag="np")
                    for j in range(nq):
                        nc.tensor.matmul(
                            pt[:, j * 256 : (j + 1) * 256],
                            lhsT=lhs[:, j * 128 : (j + 1) * 128],
                            rhs=w_sb[:],
                            start=True,
                            stop=True,
                        )
                    rows = nsb.tile([128, nq * 256], bf16, tag="rows")
                    eng = copy_rr[q % 2]
                    if eng is nc.scalar:
                        eng.copy(rows[:], pt[:])
                    else:
                        eng.tensor_copy(rows[:], pt[:])
                    rows3 = rows[:].rearrange("p (j c) -> p j c", c=256)
                    dst_e = table_e[
                        tiles[0] * 128 : tiles[0] * 128 + nq * 128
                    ].rearrange("(j p) c -> p j c", p=128)
                    dst_r = table_r[
                        tiles[0] * 128 : tiles[0] * 128 + nq * 128
                    ].rearrange("(j p) c -> p j c", p=128)
                    nc.sync.dma_start(out=dst_e, in_=rows3[:, :, 0:128])
                    nc.sync.dma_start(out=dst_r, in_=rows3[:, :, 128:256])

            def edge_phase(layer):
                t = t1m if layer == 1 else t2m
                a, bodd = t["a"], t["bodd"]

                # xr gather: 3 windows from table_r (own rows; identity idx)
                xr_all = xr3.tile([128, NCHUNK * 128], bf16, tag="xra")
                nxr = 128 if small_gather else SHARD
                nxb = 1 if small_gather else NCHUNK
                nc.gpsimd.dma_gather(
                    out_ap=xr_all[:].rearrange("p (b r) -> p b r", r=128)[:, 0:nxb, :],
                    in_ap=table_r[cfg.WB[0] :, :],
                    idxs_ap=xi_sb[:, 0:XRC],
                    num_idxs=nxr,
                    num_idxs_reg=nxr,
                    elem_size=128,
                    single_packet=False,
                    queue_num=next_q(),
                )
                for w in (1, 2):
                    tw = xr3.tile([128, NCHUNK * 128], bf16, tag="xrs")
                    nc.gpsimd.dma_gather(
                        out_ap=tw[:].rearrange("p (b r) -> p b r", r=128)[:, 0:nxb, :],
                        in_ap=table_r[cfg.WB[w] :, :],
                        idxs_ap=xi_sb[:, w * XRC : (w + 1) * XRC],
                        num_idxs=nxr,
                        num_idxs_reg=nxr,
                        elem_size=128,
                        single_packet=False,
                        queue_num=next_q(),
                    )
                    nc.vector.tensor_tensor(
                        out=xr_all[:], in0=xr_all[:], in1=tw[:], op=mybir.AluOpType.add
                    )
                xr3b = xr_all[:].rearrange("p (b r) -> p b r", r=128)

                o0 = o1 = o2 = 0
                si_rr = 0
                for gi, chs in enumerate(groups):
                    n0, n1, n2 = gsz[gi]
                    kg = int(sum(K[c] for c in chs))
                    gt = gsb.tile([128, kg * 128], bf16, tag="g")
                    g3 = gt[:].rearrange("p (b r) -> p b r", r=128)
                    blk = 0
                    w_blk = []
                    for w, kw in ((0, K0), (1, K1), (2, K2)):
                        nblk = int(sum(kw[c] for c in chs))
                        w_blk.append((blk, nblk))
                        blk += nblk
                    for w, (isb, off, nn) in enumerate(
                        ((i0_sb, o0, n0), (i1_sb, o1, n1), (i2_sb, o2, n2))
                    ):
                        b0, nblk = w_blk[w]
                        if nn == 0:
                            continue
                        nne = 128 if small_gather else nn
                        nnb = 1 if small_gather else nblk
                        nc.gpsimd.dma_gather(
                            out_ap=g3[:, b0 : b0 + nnb, :],
                            in_ap=table_e[cfg.WB[w] :, :],
                            idxs_ap=isb[:, off : off + nn // 16],
                            num_idxs=nne,
                            num_idxs_reg=nne,
                            elem_size=128,
                            single_packet=False,
                            queue_num=next_q(),
                        )
                    o0 += n0 // 16
                    o1 += n1 // 16
                    o2 += n2 // 16

                    for ci, c in enumerate(chs):
                        if no_chunk:
                            break
                        kc = int(K[c])
                        cblk = []
                        for w, kw in ((0, K0), (1, K1), (2, K2)):
                            b0 = w_blk[w][0] + int(sum(kw[cc] for cc in chs[:ci]))
                            cblk.append((b0, int(kw[c])))
                        runs = []
                        uoff = 0
                        for b0, nb in cblk:
                            if nb == 0:
                                continue
                            runs.append((b0, nb, uoff))
                            uoff += nb

                        ut = csb.tile([128, kc * 128], bf16, tag="u")
                        u3 = ut[:].rearrange("p (b r) -> p b r", r=128)
                        xrb_c = xr3b[:, c, :]
                        for ri, (b0, nb, uo) in enumerate(runs):
                            nc.vector.tensor_tensor(
                                out=u3[:, uo : uo + nb, :],
                                in0=g3[:, b0 : b0 + nb, :],
                                in1=xrb_c.unsqueeze(1).to_broadcast([128, nb, 128]),
                                op=mybir.AluOpType.add,
                            )
                        if sim_relu:
                            nc.scalar.activation(
                                out=ut[:], in_=ut[:],
                                func=mybir.ActivationFunctionType.Relu,
                            )
                        else:
                            nc.scalar.activation(
                                out=ut[:], in_=ut[:],
                                func=mybir.ActivationFunctionType.Prelu,
                                alpha=SLOPE,
                            )
                        # fold1: f[q] = u[q] + u[q+64]
                        fu = csb.tile([128, kc * 64], bf16, tag="fu")
                        f3 = fu[:].rearrange("p (b r) -> p b r", r=64)
                        nc.vector.tensor_tensor(
                            out=f3[:],
                            in0=u3[:, :, 0:64],
                            in1=u3[:, :, 64:128],
                            op=mybir.AluOpType.add,
                        )
                        lg = csb.tile([128, 2 * kc], fp32, tag="lg")
                        nc.vector.tensor_reduce(
                            out=lg[:, 0:kc], in_=f3[:, :, 0:a],
                            axis=mybir.AxisListType.X, op=mybir.AluOpType.add,
                        )
                        nc.vector.tensor_reduce(
                            out=lg[:, kc : 2 * kc], in_=f3[:, :, a + bodd : 64],
                            axis=mybir.AxisListType.X, op=mybir.AluOpType.add,
                            negate=True,
                        )
                        nc.vector.tensor_tensor(
                            out=lg[:, 0:kc], in0=lg[:, 0:kc], in1=lg[:, kc : 2 * kc],
                            op=mybir.AluOpType.add,
                        )
                        if bodd:
                            nc.vector.tensor_tensor(
                                out=lg[:, 0:kc], in0=lg[:, 0:kc], in1=u3[:, :, a],
                                op=mybir.AluOpType.add,
                            )
                            nc.vector.tensor_tensor(
                                out=lg[:, 0:kc], in0=lg[:, 0:kc],
                                in1=u3[:, :, 64 + a],
                                op=mybir.AluOpType.subtract,
                            )
                        nc.vector.tensor_scalar_min(lg[:, 0:kc], lg[:, 0:kc], 60.0)
                        av = csb.tile([128, kc], fp32, tag="av")
                        nc.scalar.activation(
                            out=av[:], in_=lg[:, 0:kc],
                            func=mybir.ActivationFunctionType.Exp,
                        )
                        sv = csb.tile([128, 4], fp32, tag="sv")
                        nc.vector.tensor_reduce(
                            out=sv[:, 0:1], in_=av[:], axis=mybir.AxisListType.X,
                            op=mybir.AluOpType.add,
                        )
                        nc.vector.tensor_scalar_max(sv[:, 1:2], sv[:, 0:1], 1e-16)
                        nc.vector.reciprocal(sv[:, 2:3], sv[:, 1:2])
                        # out-path: psum += ident.T @ (av[s] * zl_s)
                        opsum = ps2.tile([128, 128], fp32, tag="op")
                        si = 0
                        scale_rr = ("v", "a", "v")
                        for b0, nb, uo in runs:
                            for bb in range(b0, b0 + nb):
                                avs = csb.tile([128, 128], bf16, tag="avs")
                                which = scale_rr[si_rr % 3]
                                si_rr += 1
                                if which == "a":
                                    nc.scalar.activation(
                                        out=avs[:], in_=g3[:, bb, :],
                                        func=mybir.ActivationFunctionType.Copy,
                                        scale=av[:, si : si + 1],
                                    )
                                else:
                                    eng = nc.vector if which == "v" else nc.gpsimd
                                    eng.tensor_scalar(
                                        out=avs[:], in0=g3[:, bb, :],
                                        scalar1=av[:, si : si + 1], scalar2=None,
                                        op0=mybir.AluOpType.mult,
                                    )
                                nc.tensor.matmul(
                                    opsum[:], lhsT=ident, rhs=avs[:],
                                    start=(si == 0), stop=(si == kc - 1),
                                )
                                si += 1
                        # out = Relu(psum * r) in one ScalarE op
                        if layer == 1:
                            xrow = csb.tile([128, 128], bf16, tag="xrow")
                            nc.scalar.activation(
                                out=xrow[:], in_=opsum[:],
                                func=mybir.ActivationFunctionType.Relu,
                                scale=sv[:, 2:3],
                            )
                            nc.sync.dma_start(
                                out=x2s[c * 128 : (c + 1) * 128, :], in_=xrow[:]
                            )
                        else:
                            orow = csb.tile([128, 128], fp32, tag="orow")
                            nc.scalar.activation(
                                out=orow[:], in_=opsum[:],
                                func=mybir.ActivationFunctionType.Relu,
                                scale=sv[:, 2:3],
                            )
                            nc.sync.dma_start(
                                out=out_ext[c * 128 : (c + 1) * 128, :], in_=orow[:]
                            )

            def poison(layer):
                pr = pat_rows[layer - 1]
                for w in range(3):
                    z = cfg.ZROW[w]
                    nc.sync.dma_start(out=table_e[z : z + 1, :], in_=pr[0:1, :])

            for _ in range(rep):
                node_phase(1)
                poison(1)
                if no_edge:
                    continue
                edge_phase(1)
                if debug_out:
                    nc.sync.dma_start(out=dbg_x2[:], in_=x2s[:])
                    nc.sync.dma_start(out=dbg_te[:], in_=table_e[:])
                if only_layer1:
                    continue
                if not no_collective:
                    x2t = gsb.tile([128, SHARD], bf16, tag="x2t")
                    nc.sync.dma_start(out=x2t[:], in_=x2s[:], transpose=True)
                    nc.sync.dma_start(out=x2t_d[:], in_=x2t[:])
                    nc.gpsimd.collective_compute(
                        "AllGather",
                        mybir.AluOpType.bypass,
                        replica_groups=[list(range(CORES))],
                        ins=[x2t_d[:]],
                        outs=[ag[:]],
                    )
                node_phase(2)
                poison(2)
                edge_phase(2)

    nc.compile()
    return nc


# ----------------------------------------------------------------------------
# host input packing + entry point
# ----------------------------------------------------------------------------


def make_inputs(node_fts, params, pp):
    import ml_dtypes

    bf = ml_dtypes.bfloat16
    cfg = pp["cfg"]
    t1, t2 = host_transforms(params)
    nodemap = pp["nodemap"]
    nm = nodemap.reshape(-1)
    valid = nm >= 0
    x_rows = np.zeros((cfg.NPOS, IN), dtype=np.float64)
    x_rows[valid] = np.asarray(node_fts, np.float64)[nm[valid]]
    xT = np.ascontiguousarray(x_rows.T).astype(bf)

    cons = np.concatenate(
        [
            np.eye(128, dtype=np.float64),
            np.tile(t1["pat"][None, :], (128, 1)),
            np.tile(t2["pat"][None, :], (128, 1)),
        ],
        axis=1,
    ).astype(bf)

    K0, K1, K2, koff = pp["K0"], pp["K1"], pp["K2"], pp["koff"]
    SHARD = cfg.SHARD
    groups = make_groups(pp["K"])
    idx_rows = pp["idx_rows"]

    in_maps = []
    for core in range(CORES):
        i0l, i1l, i2l = [], [], []
        for chs in groups:
            f0, f1, f2 = [], [], []
            for c in chs:
                b = koff[c]
                f0.append(idx_rows[core, b : b + K0[c], :] - cfg.WB[0])
                f1.append(
                    idx_rows[core, b + K0[c] : b + K0[c] + K1[c], :] - cfg.WB[1]
                )
                f2.append(
                    idx_rows[core, b + K0[c] + K1[c] : koff[c + 1], :] - cfg.WB[2]
                )
            for fl, il in ((f0, i0l), (f1, i1l), (f2, i2l)):
                flat = np.concatenate(fl).ravel() if fl else np.zeros(0, np.int64)
                if flat.size:
                    il.append(wrap_idx(flat))
        def cat(ls, need):
            if ls:
                arr = np.concatenate(ls, axis=1)
            else:
                arr = np.zeros((128, 0), np.int16)
            if arr.shape[1] < need:
                arr = np.concatenate(
                    [arr, np.zeros((128, need - arr.shape[1]), np.int16)], axis=1
                )
            return arr

        i0cols = sum(int(sum(K0[c] for c in chs)) * 8 for chs in groups)
        i1cols = sum(int(sum(K1[c] for c in chs)) * 8 for chs in groups)
        i2cols = sum(int(sum(K2[c] for c in chs)) * 8 for chs in groups)
        xi = []
        own = core * SHARD + np.arange(SHARD)
        wsel = np.full(SHARD, 2, dtype=np.int64)
        wsel[own < cfg.WE[1]] = 1
        wsel[own < cfg.WE[0]] = 0
        for w in range(3):
            vals = np.where(wsel == w, own - cfg.WB[w], cfg.ZROW[w] - cfg.WB[w])
            xi.append(wrap_idx(vals))
        in_maps.append(
            {
                "xT": xT,
                "W1c": np.asarray(t1["Wc"], np.float64).astype(bf),
                "W2c": np.asarray(t2["Wc"], np.float64).astype(bf),
                "cons": cons,
                "idx0": cat(i0l, max(i0cols, 8)),
                "idx1": cat(i1l, max(i1cols, 8)),
                "idx2": cat(i2l, max(i2cols, 8)),
                "xidx": np.concatenate(xi, axis=1),
            }
        )
    return in_maps, (t1, t2)


def postprocess(results, pp, t2):
    cfg = pp["cfg"]
    nodemap = pp["nodemap"]
    out = np.zeros((cfg.N, D), dtype=np.float32)
    inv = np.argsort(t2["perm"])
    scale = 1.0 / t2["w"][t2["perm"]]
    for core in range(CORES):
        o = np.asarray(results[core]["out"], np.float32)
        o = (o * scale[None, :].astype(np.float32))[:, inv]
        nmc = nodemap[core]
        sel = nmc >= 0
        out[nmc[sel]] = o[sel]
    return out


_CACHE = {}


def kernel(**inputs) -> np.ndarray:
    from concourse.bass_utils import run_bass_kernel_spmd

    edge_index = np.asarray(inputs["edge_index"])
    key = hash(edge_index.tobytes())
    if key not in _CACHE:
        pp = preprocess(edge_index, FULL)
        t1, t2 = host_transforms(inputs)
        nc = build_program(pp, t1, t2, rep=1)
        _CACHE[key] = (pp, nc)
    pp, nc = _CACHE[key]
    in_maps, (t1, t2) = make_inputs(inputs["node_fts"], inputs, pp)
    res = run_bass_kernel_spmd(nc, in_maps, list(range(CORES)))
    return postprocess(res.results, pp, t2)


if __name__ == "__main__":
    import reference

    inputs = {k: np.asarray(v) for k, v in reference.setup_inputs().items()}
    pp = preprocess(inputs["edge_index"], FULL)
    K = pp["K"]
    tot = int(K.sum()) * 128
    print(f"slots/core {tot} vs {FULL.E//CORES} -> overhead {tot/(FULL.E/CORES)-1:+.1%}")
    import jax

    with jax.default_device(jax.devices("cpu")[0]):
        exp = np.asarray(reference.reference(**inputs))
    got = emulate(inputs["node_fts"], inputs, pp)
    err = np.linalg.norm(got - exp) / np.linalg.norm(exp)
    print(f"numpy emulator rel err: {err:.2e}")
